# revision 18
# baseline (speedup 1.0000x reference)
"""JSD loss kernel for Trainium2 (8 NeuronCores, row-sharded SPMD).

loss[r] = beta*KL(P||M) + (1-beta)*KL(Q||M), beta=0.5, M=(P+Q)/2
        = sum_v [ p'*lp + q'*lq - m*ln(m) ],  p'=p/2, q'=q/2, m=p'+q'

Per core: 256 rows x 32000 vocab = 2 row-blocks x 16 col-chunks.
Per-chunk schedule (C=2048):
  DMA : lp -> lplq[0:C] (SP queue), lq -> lplq[C:2C] (SP or GPSIMD queue
        - transfers serialize per issuing engine in the cost model, so
        the 64MB/core is split across two issue engines)
  ACT : pq = Exp(lplq - ln2) in ONE instr over 2C, fp32r output
  PE  : m = p' + q' via fp32r identity matmuls -> PSUM
        (fp32r runs 1 cycle/row for >=256-wide outputs; exact for adds)
  ACT : logm = Ln(m)
  DVE : C-term STT: accum = sum(logm * m)   (m read from PSUM)
  DVE : A+B in one STT over both halves: accum = sum(lplq * pq)
Activation tables: Exp and Ln share one set (natural_log_exp_and_others).
The compiler's per-function first-match selection would alternate between
exp-only and ln-only sets at 1.3us per reload (74us/core of stall); we
strip the generated loads post-compile and pre-place a single load of
the combined set (verified correct on HW).

CoreSim cost model: 219.8us/core (DVE-bound: DVE ~208us busy, ACT
~172us, SP+SWDGE DMA ~164us, PE ~83us). Measured on HW via the
reps-in-program wall-clock slope: ~230us/iteration (instrument noise
+-50us; the original baseline simmed 274.7us under the same model).
Max rel err 7.3e-5 on HW.

Rejected variants (all verified on HW or in walrus): GPSIMD
scalar_tensor_tensor / tensor_scalar+accum are rejected by walrus
codegen; tensor_tensor_reduce compiles but faults the device
(NRT_EXEC_UNIT_UNRECOVERABLE); GPSIMD tensor_tensor elementwise runs
~5x slower on HW than the cost model claims (Pool-offload flavor
G_EVERY=0); ACT-accumulator Silu assist (SILU_CHUNKS) works but its
end-of-block batch serializes into a tail that eats the gain.
"""

import sys
from contextlib import ExitStack

import numpy as np

sys.path.insert(0, "/opt/trn_rl_repo")

N, V = 2048, 32000
NCORES = 8
R = N // NCORES  # rows per core = 256
P = 128  # partitions
NBLK = R // P  # row blocks per core = 2
CHUNKS = [2048] * 15 + [1280]  # 15*2048 + 1280 = 32000, PSUM-bank aligned
LN2 = 0.6931471805599453
ACT_SET_EXP_LN = 6  # natural_log_exp_and_others: has both Exp and Ln

_CACHE = {}

# tuning knobs (read at program-build time; bench sweeps override these)
# GPSIMD elementwise ucode measured ~5x slower than the cost model on real
# HW, so the Pool-offload flavor defaults OFF.
G_EVERY = 0  # every G_EVERY-th chunk routes its B-half through GPSIMD (0=off)
POOL_DMA = frozenset({0, 3, 6, 9, 12, 15})  # chunks whose lq DMA uses SWDGE
# chunks whose A+B sum is computed by the ACT accumulator via
# 0.5*sum(Silu(x)) ~= 0.5*sum(x*e^x) = sum(x*p'). Runs correctly on HW
# (rel err 6.4e-4) but the end-of-block silu batch serializes into a tail
# that eats the DVE savings: sim 222.8us vs 219.8us without, and the
# tightest HW measurement was no better. Default OFF.
SILU_CHUNKS = frozenset()


def _strip_act_table_loads(nc):
    """Replace compiler-inserted per-function table loads (which alternate
    between the exp-only and ln-only sets, 1.3us per reload) with loads
    placed only at actual function-set transitions: set 6
    (natural_log_exp_and_others) covers Exp+Ln, set 18 (silu_and_others)
    covers Silu."""
    from concourse import mybir

    SILU = mybir.ActivationFunctionType.Silu
    SET_SILU = 18

    def required_set(func):
        return SET_SILU if func == SILU else ACT_SET_EXP_LN

    for blk in nc.main_func.blocks:
        insts = blk.instructions
        i = 0
        while i < len(insts):
            inst = insts[i]
            if isinstance(inst, mybir.InstLoadActFuncSet):
                si = inst.sync_info
                assert si is None or (not si.on_wait and not si.on_update), (
                    "table load carries sync roles; cannot strip"
                )
                del insts[i]
                continue
            i += 1
    cur = None
    for blk in nc.main_func.blocks:
        insts = blk.instructions
        i = 0
        while i < len(insts):
            inst = insts[i]
            if isinstance(inst, mybir.InstActivation):
                need = required_set(inst.func)
                if need != cur:
                    load = mybir.InstLoadActFuncSet(
                        name=nc.get_next_instruction_name(), ins=[], outs=[],
                        act_func_set_id=need,
                    )
                    load.engine = mybir.EngineType.Activation
                    nc.register_instruction(load)
                    insts.insert(i, load)
                    cur = need
                    i += 1
            i += 1
    return nc


def _build_program(reps: int = 1):
    import concourse.bacc as bacc
    import concourse.tile as tile
    from concourse import mybir

    nc = bacc.Bacc(
        "TRN2",
        target_bir_lowering=False,
        debug=False,
        enable_asserts=False,
        num_devices=1,
    )
    fp32 = mybir.dt.float32
    fp32r = mybir.dt.float32r
    bf16 = mybir.dt.bfloat16
    Exp = mybir.ActivationFunctionType.Exp
    Ln = mybir.ActivationFunctionType.Ln
    Silu = mybir.ActivationFunctionType.Silu
    mult = mybir.AluOpType.mult
    add = mybir.AluOpType.add

    lp_d = nc.dram_tensor("log_p", [R, V], fp32, kind="ExternalInput")
    lq_d = nc.dram_tensor("log_q", [R, V], fp32, kind="ExternalInput")
    # host passes np.float32 bytes; fp32r is the same layout, only the PE
    # feeding mode differs
    id_d = nc.dram_tensor("ident", [P, P], fp32r, kind="ExternalInput")
    out_d = nc.dram_tensor("loss", [R, 1], fp32, kind="ExternalOutput")

    lp = lp_d.ap()
    lq = lq_d.ap()
    out = out_d.ap()

    with tile.TileContext(nc) as tc, ExitStack() as ctx:
        const = ctx.enter_context(tc.tile_pool(name="const", bufs=1))
        loads = ctx.enter_context(tc.tile_pool(name="loads", bufs=4))
        sloads = ctx.enter_context(tc.tile_pool(name="sloads", bufs=2))
        acts = ctx.enter_context(tc.tile_pool(name="acts", bufs=3))
        logms = ctx.enter_context(tc.tile_pool(name="logms", bufs=2))
        scr = ctx.enter_context(tc.tile_pool(name="scr", bufs=2))
        sjunks = ctx.enter_context(tc.tile_pool(name="sjunks", bufs=1))
        parts = ctx.enter_context(tc.tile_pool(name="parts", bufs=2))
        outs = ctx.enter_context(tc.tile_pool(name="outs", bufs=2))
        psum = ctx.enter_context(tc.tile_pool(name="psum", bufs=2, space="PSUM"))

        ident_sb = const.tile([P, P], fp32r)
        nc.sync.dma_start(out=ident_sb[:], in_=id_d.ap())
        neg_ln2 = const.tile([P, 1], fp32)
        nc.vector.memset(neg_ln2[:], -LN2)

        nch = len(CHUNKS)
        # which chunks route their lq DMA through the gpsimd queue
        pool_dma = POOL_DMA
        for rep in range(reps):
            for b in range(NBLK):
                r0 = b * P
                t_parts = parts.tile([P, nch], fp32, tag="tp")
                b_parts = parts.tile([P, nch], fp32, tag="bp")
                c_parts = parts.tile([P, nch], fp32, tag="cp")
                n_silu = max(1, len(SILU_CHUNKS))
                s_parts = parts.tile([P, n_silu], fp32, tag="sp")
                nc.vector.memset(b_parts[:], 0.0)
                nc.vector.memset(t_parts[:], 0.0)
                nc.vector.memset(s_parts[:], 0.0)
                silu_pending = []  # (lplq tile, C) emitted after the loop
                for i, C in enumerate(CHUNKS):
                    c0 = sum(CHUNKS[:i])
                    is_silu = i in SILU_CHUNKS
                    lp_pool = sloads if is_silu else loads
                    lplq = lp_pool.tile(
                        [P, 2 * 2048], fp32, tag="slplq" if is_silu else "lplq"
                    )
                    pq = acts.tile([P, 2 * 2048], fp32r, tag="pq")
                    nc.sync.dma_start(
                        out=lplq[:, 0:C], in_=lp[r0 : r0 + P, c0 : c0 + C]
                    )
                    lq_eng = nc.gpsimd if i in pool_dma else nc.sync
                    lq_eng.dma_start(
                        out=lplq[:, C : 2 * C], in_=lq[r0 : r0 + P, c0 : c0 + C]
                    )
                    # p' = exp(lp - ln2) = p/2 ; q' = q/2 (one instr over 2C)
                    nc.scalar.activation(
                        out=pq[:, 0 : 2 * C],
                        in_=lplq[:, 0 : 2 * C],
                        func=Exp,
                        bias=neg_ln2[:],
                    )
                    # m = p' + q' (fp32r identity matmuls accumulate in PSUM)
                    m_ps = psum.tile([P, 2048], fp32, tag="m")
                    for j0 in range(0, C, 512):
                        w = min(512, C - j0)
                        nc.tensor.matmul(
                            out=m_ps[:, j0 : j0 + w],
                            lhsT=ident_sb[:],
                            rhs=pq[:, j0 : j0 + w],
                            start=True,
                            stop=False,
                        )
                        nc.tensor.matmul(
                            out=m_ps[:, j0 : j0 + w],
                            lhsT=ident_sb[:],
                            rhs=pq[:, C + j0 : C + j0 + w],
                            start=False,
                            stop=True,
                        )
                    logm = logms.tile([P, 2048], fp32, tag="logm")
                    nc.scalar.activation(out=logm[:, 0:C], in_=m_ps[:, 0:C], func=Ln)

                    # C-term on DVE: accum = sum(logm * m)
                    # (tensor_tensor_reduce lowers to an ISA op that faults
                    # real HW - use the HW-proven STT form instead)
                    junk_c = scr.tile([P, 2048], bf16, tag="junkc")
                    nc.vector.scalar_tensor_tensor(
                        out=junk_c[:, 0:C],
                        in0=logm[:, 0:C],
                        scalar=1.0,
                        in1=m_ps[:, 0:C],
                        op0=mult,
                        op1=mult,
                        accum_out=c_parts[:, i : i + 1],
                    )

                    if is_silu:
                        # A+B for this chunk comes from ACT: Silu batch after
                        # the loop (grouped so the silu table loads once)
                        silu_pending.append((lplq, C))
                    elif G_EVERY == 0 or i % G_EVERY != 0:
                        # N-flavor: A+B on DVE in one STT over both halves
                        junk_ab = scr.tile([P, 2 * 2048], bf16, tag="junkab")
                        nc.vector.scalar_tensor_tensor(
                            out=junk_ab[:, 0 : 2 * C],
                            in0=lplq[:, 0 : 2 * C],
                            scalar=1.0,
                            in1=pq[:, 0 : 2 * C],
                            op0=mult,
                            op1=mult,
                            accum_out=t_parts[:, i : i + 1],
                        )
                    else:
                        # G-flavor: A on DVE, B = sum(lq*q') on GPSIMD via
                        # product + fold-halving, DVE reduces the last 256
                        junk_a = scr.tile([P, 2048], bf16, tag="junka")
                        nc.vector.scalar_tensor_tensor(
                            out=junk_a[:, 0:C],
                            in0=lplq[:, 0:C],
                            scalar=1.0,
                            in1=pq[:, 0:C],
                            op0=mult,
                            op1=mult,
                            accum_out=t_parts[:, i : i + 1],
                        )
                        junk_b = scr.tile([P, 2048], fp32, tag="junkb")
                        nc.gpsimd.tensor_tensor(
                            out=junk_b[:, 0:C],
                            in0=lplq[:, C : 2 * C],
                            in1=pq[:, C : 2 * C].bitcast(fp32),
                            op=mult,
                        )
                        folds = scr.tile([P, 1024 + 512 + 256], fp32, tag="folds")
                        h1, h2, h3 = C // 2, C // 4, C // 8
                        f1 = folds[:, 0:h1]
                        f2 = folds[:, 1024 : 1024 + h2]
                        f3 = folds[:, 1536 : 1536 + h3]
                        nc.gpsimd.tensor_tensor(
                            out=f1, in0=junk_b[:, 0:h1],
                            in1=junk_b[:, h1:C], op=add,
                        )
                        nc.gpsimd.tensor_tensor(
                            out=f2, in0=folds[:, 0:h2],
                            in1=folds[:, h2:h1], op=add,
                        )
                        nc.gpsimd.tensor_tensor(
                            out=f3, in0=folds[:, 1024 : 1024 + h3],
                            in1=folds[:, 1024 + h3 : 1024 + h2], op=add,
                        )
                        nc.vector.tensor_reduce(
                            out=b_parts[:, i : i + 1], in_=f3,
                            op=add, axis=mybir.AxisListType.X,
                        )
                # batched Silu pass: 0.5*sum(silu(x)) ~= sum(x * e^x / 2)
                for k, (lplq_s, Cs) in enumerate(silu_pending):
                    sjunk = sjunks.tile([P, 2 * 2048], bf16, tag="sjunk")
                    nc.scalar.activation(
                        out=sjunk[:, 0 : 2 * Cs],
                        in_=lplq_s[:, 0 : 2 * Cs],
                        func=Silu,
                        accum_out=s_parts[:, k : k + 1],
                    )
                tb = parts.tile([P, nch], fp32, tag="tb")
                nc.vector.tensor_add(tb[:], t_parts[:], b_parts[:])
                d_parts = parts.tile([P, nch], fp32, tag="dp")
                nc.vector.tensor_sub(d_parts[:], tb[:], c_parts[:])
                r1 = outs.tile([P, 1], fp32, tag="r1")
                nc.vector.reduce_sum(
                    out=r1[:], in_=d_parts[:], axis=mybir.AxisListType.X
                )
                s_red = outs.tile([P, 1], fp32, tag="sred")
                nc.vector.reduce_sum(
                    out=s_red[:], in_=s_parts[:], axis=mybir.AxisListType.X
                )
                loss_b = outs.tile([P, 1], fp32, tag="lossb")
                nc.vector.scalar_tensor_tensor(
                    out=loss_b[:], in0=s_red[:], scalar=0.5, in1=r1[:],
                    op0=mult, op1=add,
                )
                nc.sync.dma_start(out=out[r0 : r0 + P, :], in_=loss_b[:])

    nc.compile()
    _strip_act_table_loads(nc)
    return nc


def _get_program(reps: int = 1):
    key = ("nc", reps)
    if key not in _CACHE:
        _CACHE[key] = _build_program(reps)
    return _CACHE[key]


def kernel(log_q: np.ndarray, log_p: np.ndarray, _trace: bool = False):
    from concourse.bass_utils import run_bass_kernel_spmd

    log_q = np.ascontiguousarray(np.asarray(log_q, dtype=np.float32))
    log_p = np.ascontiguousarray(np.asarray(log_p, dtype=np.float32))
    assert log_q.shape == (N, V) and log_p.shape == (N, V)

    nc = _get_program()
    ident = np.eye(P, dtype=np.float32)
    in_maps = []
    for c in range(NCORES):
        sl = slice(c * R, (c + 1) * R)
        in_maps.append(
            {"log_p": log_p[sl], "log_q": log_q[sl], "ident": ident}
        )
    res = run_bass_kernel_spmd(
        nc, in_maps, core_ids=list(range(NCORES)), trace=_trace
    )
    _CACHE["last_results"] = res
    outs = [res.results[c]["loss"].reshape(R) for c in range(NCORES)]
    return np.concatenate(outs, axis=0).astype(np.float32)
